# revision 10
# baseline (speedup 1.0000x reference)
"""B2Bsqrt-TANDEM LSTM kernel for Trainium2 (8 NeuronCores, data-parallel over batch).

Reference computation (per core, batch shard BL=64):
  xz = einsum('btf,gfh->tgbh', x, W) + b
  per step: z = xz_t + h @ U ; i,f,o = sigmoid(z_ifo); ct = b2bsqrt(z_c)
            c = f*c + i*ct ; h = o * b2bsqrt(c)
  LayerNorm(h) then Linear -> logits (B, T, 10)

Layout: z chunks (64, 512) accumulate in PSUM as lhsT.T @ rhs with
lhsT = xT/hT (128, 64) stationary, rhs = W/U (128, 512) moving, bf16 inputs,
fp32 PSUM accumulation. h is transposed each step with PE-transpose.
LN+FC fused: logits = rsig*(h@gw - mu*u) + (ln_b@fc_w + fc_b), with
mu/sumsq from ACT accumulators.
"""

import os
import sys

sys.path.insert(0, "/opt/trn_rl_repo")

import numpy as np
import ml_dtypes

import concourse.bass as bass
import concourse.mybir as mybir
import concourse.tile as tile
from concourse import bacc
from concourse.bass_utils import run_bass_kernel_spmd
from concourse.masks import make_identity

AF = mybir.ActivationFunctionType
OP = mybir.AluOpType
BF16 = mybir.dt.bfloat16
F32 = mybir.dt.float32

N_CORES = 8
B_FULL = 512
BL = B_FULL // N_CORES  # 64 batch rows per core
T_FULL = 100
H = 1024
G4 = 4 * H  # 4096
C = 10
KC = H // 128  # 8 contraction chunks
NC = G4 // 512  # 8 n-chunks of 512
LN_EPS = 1e-5

# chunk processing order: i(0,1) f(2,3) ct(6,7) o(4,5)
CHUNK_ORDER = [0, 1, 2, 3, 6, 7, 4, 5]


def emit(ctx, nc, tc, T, with_bias):
    sing = ctx.enter_context(tc.tile_pool(name="sing", bufs=1))
    xt_pool = ctx.enter_context(tc.tile_pool(name="xt", bufs=3))
    ht_pool = ctx.enter_context(tc.tile_pool(name="ht", bufs=2))
    gp = ctx.enter_context(tc.tile_pool(name="gp", bufs=1))
    sp = ctx.enter_context(tc.tile_pool(name="sp", bufs=2))
    zp = ctx.enter_context(tc.tile_pool(name="zp", bufs=3, space="PSUM"))
    tp = ctx.enter_context(tc.tile_pool(name="tp", bufs=2, space="PSUM"))
    fp = ctx.enter_context(tc.tile_pool(name="fp", bufs=1, space="PSUM"))
    lp = ctx.enter_context(tc.tile_pool(name="lp", bufs=1, space="PSUM"))

    dW = nc.dram_tensor("Wn", [KC, 128, G4], BF16, kind="ExternalInput")
    dU = nc.dram_tensor("Un", [KC, 128, G4], BF16, kind="ExternalInput")
    dX = nc.dram_tensor("xT", [T, KC, 128, BL], BF16, kind="ExternalInput")
    dGW = nc.dram_tensor("gw", [KC, 128, C], BF16, kind="ExternalInput")
    dUB = nc.dram_tensor("ub", [BL, C], F32, kind="ExternalInput")
    dVB = nc.dram_tensor("vbb", [BL, C], F32, kind="ExternalInput")
    if with_bias:
        dBB = nc.dram_tensor("bb", [128, G4], BF16, kind="ExternalInput")
    dOUT = nc.dram_tensor("out", [BL, T * C], F32, kind="ExternalOutput")

    # --- resident weights / constants ---
    W_sb = sing.tile([128, KC, G4], BF16)
    nc.sync.dma_start(W_sb[:], dW.rearrange("k p n -> p k n"))
    U_sb = sing.tile([128, KC, G4], BF16)
    nc.sync.dma_start(U_sb[:], dU.rearrange("k p n -> p k n"))
    gw_sb = sing.tile([128, KC, C], BF16)
    nc.sync.dma_start(gw_sb[:], dGW.rearrange("k p c -> p k c"))
    ub_sb = sing.tile([BL, C], F32)
    nc.sync.dma_start(ub_sb[:], dUB[:])
    vb_sb = sing.tile([BL, C], F32)
    nc.sync.dma_start(vb_sb[:], dVB[:])
    if with_bias:
        bb_sb = sing.tile([128, G4], BF16)
        nc.sync.dma_start(bb_sb[:], dBB[:])
        ones_col = sing.tile([128, BL], BF16)
        nc.vector.memset(ones_col[:], 0.0)
        nc.vector.memset(ones_col[0:1, :], 1.0)

    id64 = sing.tile([BL, BL], BF16)
    make_identity(nc, id64[:])
    id10 = sing.tile([C, C], F32)
    make_identity(nc, id10[:])

    eps_sb = sing.tile([BL, 1], F32)
    nc.vector.memset(eps_sb[:], LN_EPS)

    c_st = sing.tile([BL, H], F32)
    nc.vector.memset(c_st[:], 0.0)
    hT0 = sing.tile([128, KC, BL], BF16)
    nc.vector.memset(hT0[:], 0.0)
    logit_acc = sing.tile([BL, T * C], F32)

    hT_prev = hT0

    for t in range(T):
        xt = xt_pool.tile([128, KC, BL], BF16, tag="xt")
        nc.sync.dma_start(xt[:], dX[t].rearrange("k p b -> p k b"))

        sig_i = gp.tile([BL, H], BF16, tag="sig_i")
        sig_f = gp.tile([BL, H], BF16, tag="sig_f")
        sig_o = gp.tile([BL, H], BF16, tag="sig_o")
        a3 = gp.tile([BL, H], BF16, tag="a3")
        sg3 = gp.tile([BL, H], BF16, tag="sg3")

        # --- z chunks: matmul accumulate, then gate nonlinearity ---
        for n in CHUNK_ORDER:
            ns = slice(n * 512, (n + 1) * 512)
            z_ps = zp.tile([BL, 512], F32, tag="z")
            for k in range(KC):
                nc.tensor.matmul(z_ps[:], xt[:, k, :], W_sb[:, k, ns],
                                 start=(k == 0), stop=False)
            if with_bias:
                nc.tensor.matmul(z_ps[:], ones_col[:], bb_sb[:, ns],
                                 start=False, stop=False)
            for k in range(KC):
                nc.tensor.matmul(z_ps[:], hT_prev[:, k, :], U_sb[:, k, ns],
                                 start=False, stop=(k == KC - 1))
            # nonlinearity straight from PSUM (sigmoid-table phase ops)
            g, j = divmod(n, 2)
            js = slice(j * 512, (j + 1) * 512)
            if g == 0:
                nc.scalar.activation(sig_i[:, js], z_ps[:], AF.Sigmoid)
            elif g == 1:
                nc.scalar.activation(sig_f[:, js], z_ps[:], AF.Sigmoid)
            elif g == 2:
                nc.scalar.activation(sig_o[:, js], z_ps[:], AF.Sigmoid)
            else:  # c~ pre-parts: |z|, sign(z)  (both live in the sigmoid table)
                nc.scalar.activation(a3[:, js], z_ps[:], AF.Abs)
                nc.scalar.activation(sg3[:, js], z_ps[:], AF.Sign)

        # --- sqrt-table phase ---
        s3 = gp.tile([BL, H], F32, tag="s3")
        nc.scalar.activation(s3[:], a3[:], AF.Sqrt, bias=1.0)  # sqrt(1+|z|)
        nc.vector.tensor_scalar(s3[:], s3[:], 1.0, None, OP.subtract)
        ctld = gp.tile([BL, H], BF16, tag="ctld")
        nc.vector.tensor_tensor(ctld[:], s3[:], sg3[:], OP.mult)

        # c = f*c + i*ct
        tmp1 = gp.tile([BL, H], F32, tag="tmp1")
        nc.vector.tensor_tensor(tmp1[:], sig_f[:], c_st[:], OP.mult)
        tmp2 = gp.tile([BL, H], BF16, tag="tmp2")
        nc.vector.tensor_tensor(tmp2[:], sig_i[:], ctld[:], OP.mult)
        nc.vector.tensor_tensor(c_st[:], tmp1[:], tmp2[:], OP.add)

        # h = o * sign(c) * (sqrt(1+|c|)-1)
        ac = gp.tile([BL, H], BF16, tag="ac")
        nc.scalar.activation(ac[:], c_st[:], AF.Abs)
        sgc = gp.tile([BL, H], BF16, tag="sgc")
        nc.scalar.activation(sgc[:], c_st[:], AF.Sign)
        sc = gp.tile([BL, H], F32, tag="sc")
        nc.scalar.activation(sc[:], ac[:], AF.Sqrt, bias=1.0)
        nc.vector.tensor_scalar(sc[:], sc[:], 1.0, None, OP.subtract)
        hsg = gp.tile([BL, H], BF16, tag="hsg")
        nc.vector.tensor_tensor(hsg[:], sc[:], sgc[:], OP.mult)
        h_bf = gp.tile([BL, H], BF16, tag="h_bf")
        nc.vector.tensor_tensor(h_bf[:], hsg[:], sig_o[:], OP.mult)

        # stats: mu, sumsq via ACT accumulators (Copy/Square live in both tables)
        scr = gp.tile([BL, H], BF16, tag="scr")
        sumh = sp.tile([BL, 1], F32, tag="sumh")
        nc.scalar.activation(scr[:], h_bf[:], AF.Copy, accum_out=sumh[:])
        sumsq = sp.tile([BL, 1], F32, tag="sumsq")
        nc.scalar.activation(scr[:], h_bf[:], AF.Square, accum_out=sumsq[:])

        # transpose h -> hT (PE transpose per 128-col block)
        hT = ht_pool.tile([128, KC, BL], BF16, tag="hT")
        for k in range(KC):
            t_ps = tp.tile([128, BL], BF16, tag="tps")
            nc.tensor.transpose(t_ps[:], h_bf[:, k * 128:(k + 1) * 128], id64[:])
            nc.vector.tensor_copy(hT[:, k, :], t_ps[:])

        # FC: raw.T = gw.T @ hT  (10, 64)
        f_ps = fp.tile([C, BL], F32, tag="fps")
        for k in range(KC):
            nc.tensor.matmul(f_ps[:], gw_sb[:, k, :], hT[:, k, :],
                             start=(k == 0), stop=(k == KC - 1))
        fc_sb = sp.tile([C, BL], F32, tag="fc_sb")
        nc.vector.tensor_copy(fc_sb[:], f_ps[:])
        l_ps = lp.tile([BL, C], F32, tag="lps")
        nc.tensor.transpose(l_ps[:], fc_sb[:], id10[:])

        # stats math: mu = sumh/H ; var = sumsq/H - mu^2 ; rsig = 1/sqrt(var+eps)
        mu = sp.tile([BL, 1], F32, tag="mu")
        nc.vector.tensor_scalar(mu[:], sumh[:], 1.0 / H, None, OP.mult)
        musq = sp.tile([BL, 1], F32, tag="musq")
        nc.vector.tensor_tensor(musq[:], mu[:], mu[:], OP.mult)
        var = sp.tile([BL, 1], F32, tag="var")
        nc.vector.tensor_scalar(var[:], sumsq[:], 1.0 / H, None, OP.mult)
        nc.vector.tensor_tensor(var[:], var[:], musq[:], OP.subtract)
        sd = sp.tile([BL, 1], F32, tag="sd")
        nc.scalar.activation(sd[:], var[:], AF.Sqrt, bias=eps_sb[:])
        rsig = sp.tile([BL, 1], F32, tag="rsig")
        nc.vector.reciprocal(rsig[:], sd[:])

        # logits = rsig*(raw - mu*u) + vbb
        t3 = sp.tile([BL, C], F32, tag="t3")
        nc.vector.tensor_scalar_mul(t3[:], ub_sb[:], mu[:])
        t4 = sp.tile([BL, C], F32, tag="t4")
        nc.vector.tensor_tensor(t4[:], l_ps[:], t3[:], OP.subtract)
        nc.vector.tensor_scalar_mul(t4[:], t4[:], rsig[:])
        nc.vector.tensor_tensor(logit_acc[:, t * C:(t + 1) * C], t4[:], vb_sb[:],
                                OP.add)

        hT_prev = hT

    nc.sync.dma_start(dOUT[:], logit_acc[:])


def build(T=T_FULL, with_bias=False):
    from contextlib import ExitStack

    nc = bacc.Bacc("TRN2", target_bir_lowering=False)
    with tile.TileContext(nc) as tc:
        with ExitStack() as ctx:
            emit(ctx, nc, tc, T, with_bias)
    nc.compile()
    return nc


def kernel(x, W, U, b, ln_g, ln_b, fc_w, fc_b, _T=None, _trace=False):
    x = np.asarray(x, dtype=np.float32)
    W = np.asarray(W, dtype=np.float32)
    U = np.asarray(U, dtype=np.float32)
    b = np.asarray(b, dtype=np.float32)
    ln_g = np.asarray(ln_g, dtype=np.float32)
    ln_b = np.asarray(ln_b, dtype=np.float32)
    fc_w = np.asarray(fc_w, dtype=np.float32)
    fc_b = np.asarray(fc_b, dtype=np.float32)

    T = _T or x.shape[1]
    with_bias = bool(np.any(b))

    W_all = np.concatenate([W[g] for g in range(4)], axis=1)  # (H, 4H)
    U_all = np.concatenate([U[g] for g in range(4)], axis=1)
    Wn = np.ascontiguousarray(
        W_all.reshape(KC, 128, G4)).astype(ml_dtypes.bfloat16)
    Un = np.ascontiguousarray(
        U_all.reshape(KC, 128, G4)).astype(ml_dtypes.bfloat16)
    gw = (ln_g[:, None] * fc_w).reshape(KC, 128, C).astype(ml_dtypes.bfloat16)
    u_vec = (ln_g @ fc_w).astype(np.float32)  # (C,)
    vb = (ln_b @ fc_w + fc_b).astype(np.float32)
    ub_b = np.broadcast_to(u_vec, (BL, C)).copy()
    vb_b = np.broadcast_to(vb, (BL, C)).copy()

    common = {"Wn": Wn, "Un": Un, "gw": gw, "ub": ub_b, "vbb": vb_b}
    if with_bias:
        b_all = np.concatenate([b[g] for g in range(4)])  # (4H,)
        bb = np.zeros((128, G4), dtype=ml_dtypes.bfloat16)
        bb[0, :] = b_all.astype(ml_dtypes.bfloat16)
        common["bb"] = bb

    in_maps = []
    for ci in range(N_CORES):
        xc = x[ci * BL:(ci + 1) * BL, :T]           # (BL, T, H)
        xT = xc.transpose(1, 2, 0)                   # (T, H, BL)
        xT = np.ascontiguousarray(xT).reshape(T, KC, 128, BL)
        in_maps.append({"xT": xT.astype(ml_dtypes.bfloat16), **common})

    nc = build(T, with_bias)
    res = run_bass_kernel_spmd(nc, in_maps, core_ids=list(range(N_CORES)),
                               trace=_trace)
    out = np.concatenate(
        [r["out"].reshape(BL, T, C) for r in res.results], axis=0)
    if _trace:
        kernel.last_exec_time_ns = res.exec_time_ns
    return out
